# revision 8
# baseline (speedup 1.0000x reference)
"""Trainium2 Bass kernel: prefix-LM causal self-attention (B=4, T=2048, D=1024, H=16).

Sharding: 8 cores = 4 batches x 2 head-groups (8 heads each).  Each core
computes QKV projection, masked attention and the output projection for its
(batch, head-group); the two partial output-projection results per batch are
summed on the host (tensor-parallel unshard).

Mask identity used on device: allowed(q, k) <=> k <= max(q, p-1), i.e.
blocked <=> (k > q) AND (k >= p).  Scores are computed transposed
S^T[k_partition, q_free] so the softmax denominator comes free from a
ones-augmented V column in the P@V matmul, and no row-max subtraction is
needed (|scores| ~ N(0,1) after the 1/sqrt(dk) scale).
"""

import sys

for _p in ("/opt/trn_rl_repo", "/opt/pypackages"):
    if _p not in sys.path:
        sys.path.append(_p)

from contextlib import ExitStack

import numpy as np

import concourse.bass as bass  # noqa: F401
import concourse.tile as tile
from concourse import bacc, mybir
from concourse.bass_utils import run_bass_kernel_spmd

F32 = mybir.dt.float32
F32R = mybir.dt.float32r
EXP = mybir.ActivationFunctionType.Exp
ALU = mybir.AluOpType

B, T, D, H, DK = 4, 2048, 1024, 16, 64
HL = 8            # heads per core
JL = HL * DK      # 512 local attention dims
NCORES = 8
NEG = -1.0e30
SCALE = 0.125     # 1/sqrt(dk)
# Static per-q-tile key extent (prefix_lengths < 1024 by construction).
KMAX = [1024, 1024, 1536, 2048]


def _emit(ctx: ExitStack, tc, y_d, xT_d, wT_d, woT_d, pfx_d):
    nc = tc.nc

    # ---- constants ----------------------------------------------------
    const = ctx.enter_context(tc.tile_pool(name="const", bufs=1))
    ones_t = const.tile([128, 512], F32)
    nc.vector.memset(ones_t, 1.0)
    # tri_all[:, i, :][r, j] = 1.0 where (128*i + r) > j else 0  (k > q)
    tri_all = const.tile([128, 4, 512], F32)
    for i in range(4):
        nc.gpsimd.affine_select(
            tri_all[:, i, :], ones_t, pattern=[[-1, 512]], base=128 * i,
            channel_multiplier=1, compare_op=ALU.is_gt, fill=0.0,
        )
    kio = const.tile([128, 1], F32)
    nc.gpsimd.iota(kio, pattern=[[1, 1]], base=0, channel_multiplier=1,
                   allow_small_or_imprecise_dtypes=True)
    p_one = const.tile([1, 1], F32)
    nc.sync.dma_start(p_one, pfx_d)
    p_bcast = const.tile([128, 1], F32)
    nc.gpsimd.partition_broadcast(p_bcast, p_one)

    # ---- resident activations -----------------------------------------
    qk_pool = ctx.enter_context(tc.tile_pool(name="qk", bufs=1))
    qk_all = qk_pool.tile([128, 8, T], F32R)      # rows: q(0-3) then k(4-7)
    v_pool = ctx.enter_context(tc.tile_pool(name="v", bufs=1))
    v_all = v_pool.tile([128, 16, HL * 65], F32R)  # per head 64 v-dims + ones col
    # set the per-head ones columns (f32r memset fails the ISA check, so
    # bit-copy 1.0f from the f32 ones tile instead)
    ones_cols = v_all.rearrange("p t (h e) -> p t h e", e=65)[:, :, :, 64:65]
    ones_src = ones_t[:, 0:128].bitcast(F32R).rearrange(
        "p (a b c) -> p a b c", a=16, b=HL)
    nc.vector.tensor_copy(ones_cols, ones_src)

    # ---- phase 1: QKV projection --------------------------------------
    with ExitStack() as p1:
        w_pool = p1.enter_context(tc.tile_pool(name="w", bufs=1))
        w_all = w_pool.tile([128, 8, 3 * JL], F32R)
        for c in range(8):
            nc.sync.dma_start(w_all[:, c, :], wT_d[c * 128:(c + 1) * 128, :])
        x_pool = p1.enter_context(tc.tile_pool(name="xt", bufs=9))
        ps1 = p1.enter_context(tc.tile_pool(name="ps1", bufs=4, space="PSUM"))

        for tt in range(4):
            t0 = tt * 512
            xts = []
            for c in range(8):
                xt = x_pool.tile([128, 512], F32R, tag="xt")
                nc.sync.dma_start(xt, xT_d[c * 128:(c + 1) * 128, t0:t0 + 512])
                xts.append(xt)
            # Q^T and K^T (output-transposed GEMM)
            for ot in range(8):
                ps = ps1.tile([128, 512], F32, tag="ps")
                for c in range(8):
                    nc.tensor.matmul(ps, w_all[:, c, ot * 128:(ot + 1) * 128],
                                     xts[c], start=(c == 0), stop=(c == 7))
                nc.vector.tensor_copy(qk_all[:, ot, t0:t0 + 512], ps.bitcast(F32R))
            # V (natural layout GEMM), strided into 65-wide head blocks
            for st in range(4):
                kt = tt * 4 + st
                ps = ps1.tile([128, 512], F32, tag="ps")
                for c in range(8):
                    nc.tensor.matmul(ps, xts[c][:, st * 128:(st + 1) * 128],
                                     w_all[:, c, 2 * JL:3 * JL],
                                     start=(c == 0), stop=(c == 7))
                dst = v_all[:, kt, :].rearrange("p (h e) -> p h e", h=HL)[:, :, 0:64]
                nc.vector.tensor_copy(dst, ps.bitcast(F32R).rearrange("p (h e) -> p h e", h=HL))

    # ---- phase 2: attention -------------------------------------------
    # allocated after phase-1 pools close so it reuses the w/xt space
    oT_pool = ctx.enter_context(tc.tile_pool(name="ot", bufs=1))
    oT_all = oT_pool.tile([128, 4, T], F32R)
    with ExitStack() as p2:
        bias_pool = p2.enter_context(tc.tile_pool(name="bias", bufs=1))
        pv_pool = p2.enter_context(tc.tile_pool(name="pv", bufs=12))
        e_pool = p2.enter_context(tc.tile_pool(name="e", bufs=6))
        dn_pool = p2.enter_context(tc.tile_pool(name="dn", bufs=2))
        rb_pool = p2.enter_context(tc.tile_pool(name="rb", bufs=2))
        s_psum = p2.enter_context(tc.tile_pool(name="ps2", bufs=2, space="PSUM"))
        o_psum = p2.enter_context(tc.tile_pool(name="po", bufs=2, space="PSUM"))

        for qt in range(4):
            q0 = qt * 512
            nk = KMAX[qt] // 128
            # per-(k-row-tile) prefix bias vectors: (row_k >= p) * NEG
            k0s = [q0 + 128 * i for i in range(4)]
            if qt == 0:
                k0s += [512, 640, 768, 896]
            pb = {}
            for k0 in k0s:
                pk = pv_pool.tile([128, 1], F32, tag="pk")
                nc.vector.tensor_scalar_add(pk, p_bcast, float(-k0))  # p - k0
                pv = pv_pool.tile([128, 1], F32, tag="pv")
                nc.vector.tensor_scalar(pv, kio, pk, NEG, ALU.is_ge, ALU.mult)
                pb[k0] = pv
            # combined diagonal bias tiles: tri(k>q) * (k>=p)*NEG
            bias_all = bias_pool.tile([128, 4, 512], F32, tag="bias")
            for i in range(4):
                nc.vector.tensor_scalar_mul(bias_all[:, i, :], tri_all[:, i, :],
                                            pb[q0 + 128 * i])

            for hp in range(4):
                hA, hB = 2 * hp, 2 * hp + 1
                oA = o_psum.tile([65, 512], F32, tag="oa")
                oB = o_psum.tile([65, 512], F32, tag="ob")
                for ki in range(nk):
                    k0 = ki * 128
                    rel = k0 - q0
                    sps = s_psum.tile([128, 2, 512], F32, tag="s")
                    qa = qk_all[0:64, hp, q0:q0 + 512]
                    qb = qk_all[64:128, hp, q0:q0 + 512]
                    ka = qk_all[0:64, 4 + hp, k0:k0 + 128]
                    kb = qk_all[64:128, 4 + hp, k0:k0 + 128]
                    nc.tensor.matmul(sps[:, 0, :], ka, qa,
                                     start=True, stop=True)
                    nc.tensor.matmul(sps[:, 1, :], kb, qb,
                                     start=True, stop=True)
                    et = e_pool.tile([128, 2, 512], F32R, tag="e")
                    if 0 <= rel <= 384:
                        bb = bias_all[:, rel // 128, :].unsqueeze(1)
                        bb = bb.broadcast_to([128, 2, 512])
                        nc.vector.tensor_tensor(sps, sps, bb, ALU.add)
                        nc.scalar.activation(et, sps, EXP, scale=SCALE)
                    elif rel > 384:  # fully above diagonal: prefix-only rows
                        nc.scalar.activation(et, sps, EXP, bias=pb[k0],
                                             scale=SCALE)
                    else:            # fully below diagonal: all allowed
                        nc.scalar.activation(et, sps, EXP, scale=SCALE)
                    va = v_all[:, ki, hA * 65:hA * 65 + 65]
                    vb = v_all[:, ki, hB * 65:hB * 65 + 65]
                    nc.tensor.matmul(oA, va, et[:, 0, :],
                                     start=(ki == 0), stop=(ki == nk - 1),
                                     skip_group_check=True)
                    nc.tensor.matmul(oB, vb, et[:, 1, :],
                                     start=(ki == 0), stop=(ki == nk - 1),
                                     skip_group_check=True)
                # normalize by the ones-row denominator and evict to O^T
                dn = dn_pool.tile([1, 1024], F32, tag="dn")
                nc.vector.tensor_copy(dn[0:1, 0:512], oA[64:65, :])
                nc.vector.tensor_copy(dn[0:1, 512:1024], oB[64:65, :])
                rv = dn_pool.tile([1, 1024], F32, tag="rv")
                nc.vector.reciprocal(rv, dn)
                rb = rb_pool.tile([128, 1024], F32, tag="rb")
                nc.gpsimd.partition_broadcast(rb, rv)
                nc.vector.tensor_tensor(oT_all[0:64, hp, q0:q0 + 512],
                                        oA[0:64, :], rb[0:64, 0:512], ALU.mult)
                nc.vector.tensor_tensor(oT_all[64:128, hp, q0:q0 + 512],
                                        oB[0:64, :], rb[64:128, 512:1024],
                                        ALU.mult)

    # ---- phase 3: output projection -----------------------------------
    with ExitStack() as p3:
        wo_pool = p3.enter_context(tc.tile_pool(name="wo", bufs=1))
        wo_all = wo_pool.tile([128, 4, D], F32R)
        for jc in range(4):
            nc.sync.dma_start(wo_all[:, jc, :], woT_d[jc * 128:(jc + 1) * 128, :])
        y_pool = p3.enter_context(tc.tile_pool(name="ysb", bufs=4))
        ps3 = p3.enter_context(tc.tile_pool(name="ps3", bufs=4, space="PSUM"))
        for ttt in range(16):
            t0 = ttt * 128
            for ob in range(2):
                ps = ps3.tile([128, 512], F32, tag="y")
                for jc in range(4):
                    nc.tensor.matmul(ps, oT_all[:, jc, t0:t0 + 128],
                                     wo_all[:, jc, ob * 512:(ob + 1) * 512],
                                     start=(jc == 0), stop=(jc == 3))
                ysb = y_pool.tile([128, 512], F32, tag="ysb")
                nc.vector.tensor_copy(ysb, ps)
                nc.sync.dma_start(y_d[t0:t0 + 128, ob * 512:(ob + 1) * 512], ysb)


def build_module():
    nc = bacc.Bacc("TRN2", target_bir_lowering=False, debug=False,
                   num_devices=NCORES)
    xT_d = nc.dram_tensor("xT", [D, T], F32R, kind="ExternalInput").ap()
    wT_d = nc.dram_tensor("wT", [D, 3 * JL], F32R, kind="ExternalInput").ap()
    woT_d = nc.dram_tensor("woT", [JL, D], F32R, kind="ExternalInput").ap()
    pfx_d = nc.dram_tensor("pfx", [1, 1], F32, kind="ExternalInput").ap()
    y_d = nc.dram_tensor("y", [T, D], F32, kind="ExternalOutput").ap()
    with tile.TileContext(nc) as tc:
        with ExitStack() as ctx:
            _emit(ctx, tc, y_d, xT_d, wT_d, woT_d, pfx_d)
    nc.compile()
    return nc


_NC = None


def _get_nc():
    global _NC
    if _NC is None:
        _NC = build_module()
    return _NC


def shard_inputs(x, prefix_lengths, W_qkv, W_o):
    x = np.asarray(x, dtype=np.float32)
    W_qkv = np.asarray(W_qkv, dtype=np.float32)
    W_o = np.asarray(W_o, dtype=np.float32)
    pl = np.asarray(prefix_lengths)
    in_maps = []
    for c in range(NCORES):
        b, g = c // 2, c % 2
        W_loc = np.concatenate([
            W_qkv[JL * g:JL * (g + 1)],
            W_qkv[D + JL * g:D + JL * (g + 1)],
            W_qkv[2 * D + JL * g:2 * D + JL * (g + 1)],
        ], axis=0)
        p = float(min(max(int(pl[b]), 0), T))
        in_maps.append({
            "xT": np.ascontiguousarray(x[b].T),
            "wT": np.ascontiguousarray(W_loc.T),
            "woT": np.ascontiguousarray(W_o[:, JL * g:JL * (g + 1)].T),
            "pfx": np.array([[p]], dtype=np.float32),
        })
    return in_maps


def run(x, prefix_lengths, W_qkv, W_o, **kw):
    """Run the kernel; returns (y, BassKernelResults)."""
    nc = _get_nc()
    in_maps = shard_inputs(x, prefix_lengths, W_qkv, W_o)
    res = run_bass_kernel_spmd(nc, in_maps, core_ids=list(range(NCORES)), **kw)
    y = np.zeros((B, T, D), dtype=np.float32)
    for c in range(NCORES):
        y[c // 2] += res.results[c]["y"]
    return y, res


def kernel(x, prefix_lengths, W_qkv, W_o):
    y, _ = run(x, prefix_lengths, W_qkv, W_o)
    return y


# revision 11
# speedup vs baseline: 1.2010x; 1.2010x over previous
"""Trainium2 Bass kernel: prefix-LM causal self-attention (B=4, T=2048, D=1024, H=16).

Sharding: 8 cores = 4 batches x 2 head-groups (8 heads each).  Each core
computes QKV projection, masked attention and the output projection for its
(batch, head-group); the two partial output-projection results per batch are
summed on the host (tensor-parallel unshard).

Mask identity used on device: allowed(q, k) <=> k <= max(q, p-1), i.e.
blocked <=> (k > q) AND (k >= p).  Scores are computed transposed
S^T[k_partition, q_free] so the softmax denominator comes free from a
ones-augmented V column in the P@V matmul, and no row-max subtraction is
needed (|scores| ~ N(0,1) after the 1/sqrt(dk) scale).
"""

import sys

for _p in ("/opt/trn_rl_repo", "/opt/pypackages"):
    if _p not in sys.path:
        sys.path.append(_p)

from contextlib import ExitStack

import numpy as np

import concourse.bass as bass  # noqa: F401
import concourse.tile as tile
from concourse import bacc, mybir
from concourse.bass_utils import run_bass_kernel_spmd

F32 = mybir.dt.float32
F32R = mybir.dt.float32r
EXP = mybir.ActivationFunctionType.Exp
ALU = mybir.AluOpType

B, T, D, H, DK = 4, 2048, 1024, 16, 64
HL = 8            # heads per core
JL = HL * DK      # 512 local attention dims
NCORES = 8
NEG = -1.0e30
SCALE = 0.125     # 1/sqrt(dk)
# Static per-q-tile key extent (prefix_lengths < 1024 by construction).
KMAX = [1024, 1024, 1536, 2048]


def _emit(ctx: ExitStack, tc, y_d, xT_d, wT_d, woT_d, pfx_d):
    nc = tc.nc

    # ---- constants ----------------------------------------------------
    const = ctx.enter_context(tc.tile_pool(name="const", bufs=1))
    ones_t = const.tile([128, 512], F32)
    nc.vector.memset(ones_t, 1.0)
    # tri_all[:, i, :][r, j] = 1.0 where (128*i + r) > j else 0  (k > q)
    tri_all = const.tile([128, 4, 512], F32)
    for i in range(4):
        nc.gpsimd.affine_select(
            tri_all[:, i, :], ones_t, pattern=[[-1, 512]], base=128 * i,
            channel_multiplier=1, compare_op=ALU.is_gt, fill=0.0,
        )
    kio = const.tile([128, 1], F32)
    nc.gpsimd.iota(kio, pattern=[[1, 1]], base=0, channel_multiplier=1,
                   allow_small_or_imprecise_dtypes=True)
    p_one = const.tile([1, 1], F32)
    nc.sync.dma_start(p_one, pfx_d)
    p_bcast = const.tile([128, 1], F32)
    nc.gpsimd.partition_broadcast(p_bcast, p_one)

    # ---- resident activations -----------------------------------------
    qk_pool = ctx.enter_context(tc.tile_pool(name="qk", bufs=1))
    qk_all = qk_pool.tile([128, 8, T], F32R)      # rows: q(0-3) then k(4-7)
    v_pool = ctx.enter_context(tc.tile_pool(name="v", bufs=1))
    v_all = v_pool.tile([128, 16, HL * 65], F32R)  # per head 64 v-dims + ones col
    # set the per-head ones columns (f32r memset fails the ISA check, so
    # bit-copy 1.0f from the f32 ones tile instead)
    ones_cols = v_all.rearrange("p t (h e) -> p t h e", e=65)[:, :, :, 64:65]
    ones_src = ones_t[:, 0:128].bitcast(F32R).rearrange(
        "p (a b c) -> p a b c", a=16, b=HL)
    nc.vector.tensor_copy(ones_cols, ones_src)

    # ---- phase 1: QKV projection --------------------------------------
    with ExitStack() as p1:
        w_pool = p1.enter_context(tc.tile_pool(name="w", bufs=1))
        w_all = w_pool.tile([128, 8, 3 * JL], F32R)
        x_pool = p1.enter_context(tc.tile_pool(name="xt", bufs=9))
        ps1 = p1.enter_context(tc.tile_pool(name="ps1", bufs=4, space="PSUM"))

        for tt in range(4):
            t0 = tt * 512
            xts = []
            for c in range(8):
                xt = x_pool.tile([128, 512], F32R, tag="xt")
                nc.sync.dma_start(xt, xT_d[c * 128:(c + 1) * 128, t0:t0 + 512])
                xts.append(xt)
            if tt == 0:
                # weights stream in behind the first x tiles
                for c in range(8):
                    nc.sync.dma_start(w_all[:, c, :],
                                      wT_d[c * 128:(c + 1) * 128, :])
            # Q^T and K^T (output-transposed GEMM)
            for ot in range(8):
                ps = ps1.tile([128, 512], F32, tag="ps")
                for c in range(8):
                    nc.tensor.matmul(ps, w_all[:, c, ot * 128:(ot + 1) * 128],
                                     xts[c], start=(c == 0), stop=(c == 7))
                nc.vector.tensor_copy(qk_all[:, ot, t0:t0 + 512], ps.bitcast(F32R))
            # V (natural layout GEMM), strided into 65-wide head blocks
            for st in range(4):
                kt = tt * 4 + st
                ps = ps1.tile([128, 512], F32, tag="ps")
                for c in range(8):
                    nc.tensor.matmul(ps, xts[c][:, st * 128:(st + 1) * 128],
                                     w_all[:, c, 2 * JL:3 * JL],
                                     start=(c == 0), stop=(c == 7))
                dst = v_all[:, kt, :].rearrange("p (h e) -> p h e", h=HL)[:, :, 0:64]
                nc.vector.tensor_copy(dst, ps.bitcast(F32R).rearrange("p (h e) -> p h e", h=HL))

    # ---- phase 2: attention -------------------------------------------
    # allocated after phase-1 pools close so it reuses the w/xt space
    oT_pool = ctx.enter_context(tc.tile_pool(name="ot", bufs=1))
    oT_all = oT_pool.tile([128, 4, T], F32R)
    with ExitStack() as p2:
        bias_pool = p2.enter_context(tc.tile_pool(name="bias", bufs=1))
        pv_pool = p2.enter_context(tc.tile_pool(name="pv", bufs=12))
        e_pool = p2.enter_context(tc.tile_pool(name="e", bufs=6))
        dn_pool = p2.enter_context(tc.tile_pool(name="dn", bufs=2))
        rb_pool = p2.enter_context(tc.tile_pool(name="rb", bufs=2))
        s_psum = p2.enter_context(tc.tile_pool(name="ps2", bufs=3, space="PSUM"))
        o_psum = p2.enter_context(tc.tile_pool(name="po", bufs=1, space="PSUM"))

        for qt in range(4):
            q0 = qt * 512
            nk = KMAX[qt] // 128
            # per-(k-row-tile) prefix bias vectors: (row_k >= p) * NEG
            k0s = [q0 + 128 * i for i in range(4)]
            if qt == 0:
                k0s += [512, 640, 768, 896]
            pb = {}
            for k0 in k0s:
                pk = pv_pool.tile([128, 1], F32, tag="pk")
                nc.vector.tensor_scalar_add(pk, p_bcast, float(-k0))  # p - k0
                pv = pv_pool.tile([128, 1], F32, tag="pv")
                nc.vector.tensor_scalar(pv, kio, pk, NEG, ALU.is_ge, ALU.mult)
                pb[k0] = pv
            # combined diagonal bias tiles: tri(k>q) * (k>=p)*NEG
            bias_all = bias_pool.tile([128, 4, 512], F32, tag="bias")
            for i in range(4):
                nc.vector.tensor_scalar_mul(bias_all[:, i, :], tri_all[:, i, :],
                                            pb[q0 + 128 * i])

            # software-pipelined over a flat (hp, ki) step list: S matmuls
            # are issued PIPE steps ahead of the mask/exp/V consumers so the
            # in-order PE queue never stalls on the DVE/ACT exp chain.
            steps = [(hp, ki) for hp in range(4) for ki in range(nk)]
            PIPE = 2
            live = {}
            otiles = {}

            def emit_s(j):
                hp, ki = steps[j]
                k0 = ki * 128
                sps = s_psum.tile([128, 2, 512], F32, tag="s")
                qa = qk_all[0:64, hp, q0:q0 + 512]
                qb = qk_all[64:128, hp, q0:q0 + 512]
                ka = qk_all[0:64, 4 + hp, k0:k0 + 128]
                kb = qk_all[64:128, 4 + hp, k0:k0 + 128]
                nc.tensor.matmul(sps[:, 0, :], ka, qa, start=True, stop=True)
                nc.tensor.matmul(sps[:, 1, :], kb, qb, start=True, stop=True)
                live[j] = sps

            def emit_v(j):
                hp, ki = steps[j]
                hA, hB = 2 * hp, 2 * hp + 1
                k0 = ki * 128
                rel = k0 - q0
                sps = live.pop(j)
                et = e_pool.tile([128, 2, 512], F32R, tag="e")
                if 0 <= rel <= 384:
                    bb = bias_all[:, rel // 128, :].unsqueeze(1)
                    bb = bb.broadcast_to([128, 2, 512])
                    nc.vector.tensor_tensor(sps, sps, bb, ALU.add)
                    nc.scalar.activation(et, sps, EXP, scale=SCALE)
                elif rel > 384:      # fully above diagonal: prefix-only rows
                    nc.scalar.activation(et, sps, EXP, bias=pb[k0], scale=SCALE)
                else:                # fully below diagonal: all allowed
                    nc.scalar.activation(et, sps, EXP, scale=SCALE)
                if ki == 0:
                    otiles[hp] = (
                        o_psum.tile([65, 512], F32, tag="oa", name=f"oa{hp}"),
                        o_psum.tile([65, 512], F32, tag="ob", name=f"ob{hp}"),
                    )
                oA, oB = otiles[hp]
                va = v_all[:, ki, hA * 65:hA * 65 + 65]
                vb = v_all[:, ki, hB * 65:hB * 65 + 65]
                nc.tensor.matmul(oA, va, et[:, 0, :],
                                 start=(ki == 0), stop=(ki == nk - 1),
                                 skip_group_check=True)
                nc.tensor.matmul(oB, vb, et[:, 1, :],
                                 start=(ki == 0), stop=(ki == nk - 1),
                                 skip_group_check=True)
                if ki == nk - 1:
                    # normalize by the ones-row denominator, evict to O^T
                    dn = dn_pool.tile([1, 1024], F32, tag="dn")
                    nc.vector.tensor_copy(dn[0:1, 0:512], oA[64:65, :])
                    nc.vector.tensor_copy(dn[0:1, 512:1024], oB[64:65, :])
                    rv = dn_pool.tile([1, 1024], F32, tag="rv")
                    nc.vector.reciprocal_approx_fast(out=rv, in_=dn)
                    rb = rb_pool.tile([128, 1024], F32, tag="rb")
                    nc.gpsimd.partition_broadcast(rb, rv)
                    nc.vector.tensor_tensor(oT_all[0:64, hp, q0:q0 + 512],
                                            oA[0:64, :], rb[0:64, 0:512],
                                            ALU.mult)
                    nc.vector.tensor_tensor(oT_all[64:128, hp, q0:q0 + 512],
                                            oB[0:64, :], rb[64:128, 512:1024],
                                            ALU.mult)

            for j in range(len(steps) + PIPE):
                if j < len(steps):
                    emit_s(j)
                if j >= PIPE:
                    emit_v(j - PIPE)

    # ---- phase 3: output projection -----------------------------------
    with ExitStack() as p3:
        wo_pool = p3.enter_context(tc.tile_pool(name="wo", bufs=1))
        wo_all = wo_pool.tile([128, 4, D], F32R)
        for jc in range(4):
            nc.sync.dma_start(wo_all[:, jc, :], woT_d[jc * 128:(jc + 1) * 128, :])
        y_pool = p3.enter_context(tc.tile_pool(name="ysb", bufs=4))
        ps3 = p3.enter_context(tc.tile_pool(name="ps3", bufs=4, space="PSUM"))
        for ttt in range(16):
            t0 = ttt * 128
            for ob in range(2):
                ps = ps3.tile([128, 512], F32, tag="y")
                for jc in range(4):
                    nc.tensor.matmul(ps, oT_all[:, jc, t0:t0 + 128],
                                     wo_all[:, jc, ob * 512:(ob + 1) * 512],
                                     start=(jc == 0), stop=(jc == 3))
                ysb = y_pool.tile([128, 512], F32, tag="ysb")
                nc.vector.tensor_copy(ysb, ps)
                nc.sync.dma_start(y_d[t0:t0 + 128, ob * 512:(ob + 1) * 512], ysb)


def build_module():
    nc = bacc.Bacc("TRN2", target_bir_lowering=False, debug=False,
                   num_devices=NCORES)
    xT_d = nc.dram_tensor("xT", [D, T], F32R, kind="ExternalInput").ap()
    wT_d = nc.dram_tensor("wT", [D, 3 * JL], F32R, kind="ExternalInput").ap()
    woT_d = nc.dram_tensor("woT", [JL, D], F32R, kind="ExternalInput").ap()
    pfx_d = nc.dram_tensor("pfx", [1, 1], F32, kind="ExternalInput").ap()
    y_d = nc.dram_tensor("y", [T, D], F32, kind="ExternalOutput").ap()
    with tile.TileContext(nc) as tc:
        with ExitStack() as ctx:
            _emit(ctx, tc, y_d, xT_d, wT_d, woT_d, pfx_d)
    nc.compile()
    return nc


_NC = None


def _get_nc():
    global _NC
    if _NC is None:
        _NC = build_module()
    return _NC


def shard_inputs(x, prefix_lengths, W_qkv, W_o):
    x = np.asarray(x, dtype=np.float32)
    W_qkv = np.asarray(W_qkv, dtype=np.float32)
    W_o = np.asarray(W_o, dtype=np.float32)
    pl = np.asarray(prefix_lengths)
    in_maps = []
    for c in range(NCORES):
        b, g = c // 2, c % 2
        W_loc = np.concatenate([
            W_qkv[JL * g:JL * (g + 1)],
            W_qkv[D + JL * g:D + JL * (g + 1)],
            W_qkv[2 * D + JL * g:2 * D + JL * (g + 1)],
        ], axis=0)
        p = float(min(max(int(pl[b]), 0), T))
        in_maps.append({
            "xT": np.ascontiguousarray(x[b].T),
            "wT": np.ascontiguousarray(W_loc.T),
            "woT": np.ascontiguousarray(W_o[:, JL * g:JL * (g + 1)].T),
            "pfx": np.array([[p]], dtype=np.float32),
        })
    return in_maps


def run(x, prefix_lengths, W_qkv, W_o, **kw):
    """Run the kernel; returns (y, BassKernelResults)."""
    nc = _get_nc()
    in_maps = shard_inputs(x, prefix_lengths, W_qkv, W_o)
    res = run_bass_kernel_spmd(nc, in_maps, core_ids=list(range(NCORES)), **kw)
    y = np.zeros((B, T, D), dtype=np.float32)
    for c in range(NCORES):
        y[c // 2] += res.results[c]["y"]
    return y, res


def kernel(x, prefix_lengths, W_qkv, W_o):
    y, _ = run(x, prefix_lengths, W_qkv, W_o)
    return y


# revision 12
# speedup vs baseline: 1.2148x; 1.0115x over previous
"""Trainium2 Bass kernel: prefix-LM causal self-attention (B=4, T=2048, D=1024, H=16).

Sharding: 8 cores = 4 batches x 2 head-groups (8 heads each).  Each core
computes QKV projection, masked attention and the output projection for its
(batch, head-group); the two partial output-projection results per batch are
summed on the host (tensor-parallel unshard).

Mask identity used on device: allowed(q, k) <=> k <= max(q, p-1), i.e.
blocked <=> (k > q) AND (k >= p).  Scores are computed transposed
S^T[k_partition, q_free] so the softmax denominator comes free from a
ones-augmented V column in the P@V matmul, and no row-max subtraction is
needed (|scores| ~ N(0,1) after the 1/sqrt(dk) scale).
"""

import sys

for _p in ("/opt/trn_rl_repo", "/opt/pypackages"):
    if _p not in sys.path:
        sys.path.append(_p)

from contextlib import ExitStack

import numpy as np

import concourse.bass as bass  # noqa: F401
import concourse.tile as tile
from concourse import bacc, mybir
from concourse.bass_utils import run_bass_kernel_spmd

F32 = mybir.dt.float32
F32R = mybir.dt.float32r
EXP = mybir.ActivationFunctionType.Exp
ALU = mybir.AluOpType

B, T, D, H, DK = 4, 2048, 1024, 16, 64
HL = 8            # heads per core
JL = HL * DK      # 512 local attention dims
NCORES = 8
NEG = -1.0e30
SCALE = 0.125     # 1/sqrt(dk)
# Static per-q-tile key extent (prefix_lengths < 1024 by construction).
KMAX = [1024, 1024, 1536, 2048]


def _emit(ctx: ExitStack, tc, y_d, xT_d, wT_d, woT_d, pfx_d):
    nc = tc.nc

    # ---- constants ----------------------------------------------------
    const = ctx.enter_context(tc.tile_pool(name="const", bufs=1))
    ones_t = const.tile([128, 512], F32)
    nc.vector.memset(ones_t, 1.0)
    # tri_all[:, i, :][r, j] = 1.0 where (128*i + r) > j else 0  (k > q)
    tri_all = const.tile([128, 4, 512], F32)
    for i in range(4):
        nc.gpsimd.affine_select(
            tri_all[:, i, :], ones_t, pattern=[[-1, 512]], base=128 * i,
            channel_multiplier=1, compare_op=ALU.is_gt, fill=0.0,
        )
    kio = const.tile([128, 1], F32)
    nc.gpsimd.iota(kio, pattern=[[1, 1]], base=0, channel_multiplier=1,
                   allow_small_or_imprecise_dtypes=True)
    p_one = const.tile([1, 1], F32)
    nc.sync.dma_start(p_one, pfx_d)
    p_bcast = const.tile([128, 1], F32)
    nc.gpsimd.partition_broadcast(p_bcast, p_one)

    # ---- resident activations -----------------------------------------
    qk_pool = ctx.enter_context(tc.tile_pool(name="qk", bufs=1))
    qk_all = qk_pool.tile([128, 8, T], F32R)      # rows: q(0-3) then k(4-7)
    v_pool = ctx.enter_context(tc.tile_pool(name="v", bufs=1))
    v_all = v_pool.tile([128, 16, HL * 65], F32R)  # per head 64 v-dims + ones col
    # set the per-head ones columns (f32r memset fails the ISA check, so
    # bit-copy 1.0f from the f32 ones tile instead)
    ones_cols = v_all.rearrange("p t (h e) -> p t h e", e=65)[:, :, :, 64:65]
    ones_src = ones_t[:, 0:128].bitcast(F32R).rearrange(
        "p (a b c) -> p a b c", a=16, b=HL)
    nc.vector.tensor_copy(ones_cols, ones_src)

    # ---- phase 1: QKV projection --------------------------------------
    with ExitStack() as p1:
        w_pool = p1.enter_context(tc.tile_pool(name="w", bufs=1))
        w_tiles = [w_pool.tile([128, 3 * JL], F32R, name=f"w{c}", tag=f"w{c}")
                   for c in range(8)]
        x_pool = p1.enter_context(tc.tile_pool(name="xt", bufs=9))
        ps1 = p1.enter_context(tc.tile_pool(name="ps1", bufs=4, space="PSUM"))

        for tt in range(4):
            t0 = tt * 512
            xts = []
            for c in range(8):
                xt = x_pool.tile([128, 512], F32R, tag="xt")
                nc.sync.dma_start(xt, xT_d[c * 128:(c + 1) * 128, t0:t0 + 512])
                xts.append(xt)
            if tt == 0:
                # weights stream in behind the first x tiles
                for c in range(8):
                    nc.sync.dma_start(w_tiles[c],
                                      wT_d[c * 128:(c + 1) * 128, :])
            # Q^T and K^T (output-transposed GEMM)
            for ot in range(8):
                ps = ps1.tile([128, 512], F32, tag="ps")
                for c in range(8):
                    nc.tensor.matmul(ps, w_tiles[c][:, ot * 128:(ot + 1) * 128],
                                     xts[c], start=(c == 0), stop=(c == 7))
                nc.vector.tensor_copy(qk_all[:, ot, t0:t0 + 512], ps.bitcast(F32R))
            # V (natural layout GEMM), strided into 65-wide head blocks
            for st in range(4):
                kt = tt * 4 + st
                ps = ps1.tile([128, 512], F32, tag="ps")
                for c in range(8):
                    nc.tensor.matmul(ps, xts[c][:, st * 128:(st + 1) * 128],
                                     w_tiles[c][:, 2 * JL:3 * JL],
                                     start=(c == 0), stop=(c == 7))
                dst = v_all[:, kt, :].rearrange("p (h e) -> p h e", h=HL)[:, :, 0:64]
                nc.vector.tensor_copy(dst, ps.bitcast(F32R).rearrange("p (h e) -> p h e", h=HL))

    # ---- phase 2: attention -------------------------------------------
    # allocated after phase-1 pools close so it reuses the w/xt space
    oT_pool = ctx.enter_context(tc.tile_pool(name="ot", bufs=1))
    oT_all = oT_pool.tile([128, 4, T], F32R)
    with ExitStack() as p2:
        bias_pool = p2.enter_context(tc.tile_pool(name="bias", bufs=1))
        pv_pool = p2.enter_context(tc.tile_pool(name="pv", bufs=12))
        e_pool = p2.enter_context(tc.tile_pool(name="e", bufs=6))
        dn_pool = p2.enter_context(tc.tile_pool(name="dn", bufs=2))
        rb_pool = p2.enter_context(tc.tile_pool(name="rb", bufs=2))
        s_psum = p2.enter_context(tc.tile_pool(name="ps2", bufs=3, space="PSUM"))
        o_psum = p2.enter_context(tc.tile_pool(name="po", bufs=1, space="PSUM"))

        for qt in range(4):
            q0 = qt * 512
            nk = KMAX[qt] // 128
            # per-(k-row-tile) prefix bias vectors: (row_k >= p) * NEG
            k0s = [q0 + 128 * i for i in range(4)]
            if qt == 0:
                k0s += [512, 640, 768, 896]
            pb = {}
            for k0 in k0s:
                pk = pv_pool.tile([128, 1], F32, tag="pk")
                nc.vector.tensor_scalar_add(pk, p_bcast, float(-k0))  # p - k0
                pv = pv_pool.tile([128, 1], F32, tag="pv")
                nc.vector.tensor_scalar(pv, kio, pk, NEG, ALU.is_ge, ALU.mult)
                pb[k0] = pv
            # combined diagonal bias tiles: tri(k>q) * (k>=p)*NEG
            bias_all = bias_pool.tile([128, 4, 512], F32, tag="bias")
            for i in range(4):
                nc.vector.tensor_scalar_mul(bias_all[:, i, :], tri_all[:, i, :],
                                            pb[q0 + 128 * i])

            # software-pipelined over a flat (hp, ki) step list: S matmuls
            # are issued PIPE steps ahead of the mask/exp/V consumers so the
            # in-order PE queue never stalls on the DVE/ACT exp chain.
            def kt_order(nk):
                # diagonal (DVE-masked) k-tiles sit at rel 0..384; spread
                # them out so their longer mask+exp chains are hidden by
                # cheap below/above-diagonal steps
                diag = [k for k in range(nk) if 0 <= k * 128 - q0 <= 384]
                rest = [k for k in range(nk) if k not in diag]
                out, di, ri = [], 0, 0
                for j in range(nk):
                    if j % 4 == 0 and di < len(diag):
                        out.append(diag[di]); di += 1
                    elif ri < len(rest):
                        out.append(rest[ri]); ri += 1
                    else:
                        out.append(diag[di]); di += 1
                return out

            korder = kt_order(nk)
            steps = [(hp, ki) for hp in range(4) for ki in korder]
            PIPE = 2
            live = {}
            otiles = {}

            def emit_s(j):
                hp, ki = steps[j]
                k0 = ki * 128
                sps = s_psum.tile([128, 2, 512], F32, tag="s")
                qa = qk_all[0:64, hp, q0:q0 + 512]
                qb = qk_all[64:128, hp, q0:q0 + 512]
                ka = qk_all[0:64, 4 + hp, k0:k0 + 128]
                kb = qk_all[64:128, 4 + hp, k0:k0 + 128]
                nc.tensor.matmul(sps[:, 0, :], ka, qa, start=True, stop=True)
                nc.tensor.matmul(sps[:, 1, :], kb, qb, start=True, stop=True)
                live[j] = sps

            def emit_v(j):
                hp, ki = steps[j]
                pos = j % nk
                hA, hB = 2 * hp, 2 * hp + 1
                k0 = ki * 128
                rel = k0 - q0
                sps = live.pop(j)
                et = e_pool.tile([128, 2, 512], F32R, tag="e")
                if 0 <= rel <= 384:
                    bb = bias_all[:, rel // 128, :].unsqueeze(1)
                    bb = bb.broadcast_to([128, 2, 512])
                    nc.vector.tensor_tensor(sps, sps, bb, ALU.add)
                    nc.scalar.activation(et, sps, EXP, scale=SCALE)
                elif rel > 384:      # fully above diagonal: prefix-only rows
                    nc.scalar.activation(et, sps, EXP, bias=pb[k0], scale=SCALE)
                else:                # fully below diagonal: all allowed
                    nc.scalar.activation(et, sps, EXP, scale=SCALE)
                if pos == 0:
                    otiles[hp] = (
                        o_psum.tile([65, 512], F32, tag="oa", name=f"oa{hp}"),
                        o_psum.tile([65, 512], F32, tag="ob", name=f"ob{hp}"),
                    )
                oA, oB = otiles[hp]
                va = v_all[:, ki, hA * 65:hA * 65 + 65]
                vb = v_all[:, ki, hB * 65:hB * 65 + 65]
                nc.tensor.matmul(oA, va, et[:, 0, :],
                                 start=(pos == 0), stop=(pos == nk - 1),
                                 skip_group_check=True)
                nc.tensor.matmul(oB, vb, et[:, 1, :],
                                 start=(pos == 0), stop=(pos == nk - 1),
                                 skip_group_check=True)
                if pos == nk - 1:
                    # normalize by the ones-row denominator, evict to O^T
                    dn = dn_pool.tile([1, 1024], F32, tag="dn")
                    nc.vector.tensor_copy(dn[0:1, 0:512], oA[64:65, :])
                    nc.vector.tensor_copy(dn[0:1, 512:1024], oB[64:65, :])
                    rv = dn_pool.tile([1, 1024], F32, tag="rv")
                    nc.vector.reciprocal_approx_fast(out=rv, in_=dn)
                    rb = rb_pool.tile([128, 1024], F32, tag="rb")
                    nc.gpsimd.partition_broadcast(rb, rv)
                    nc.vector.tensor_tensor(oT_all[0:64, hp, q0:q0 + 512],
                                            oA[0:64, :], rb[0:64, 0:512],
                                            ALU.mult)
                    nc.vector.tensor_tensor(oT_all[64:128, hp, q0:q0 + 512],
                                            oB[0:64, :], rb[64:128, 512:1024],
                                            ALU.mult)

            for j in range(len(steps) + PIPE):
                if j < len(steps):
                    emit_s(j)
                if j >= PIPE:
                    emit_v(j - PIPE)

    # ---- phase 3: output projection -----------------------------------
    with ExitStack() as p3:
        wo_pool = p3.enter_context(tc.tile_pool(name="wo", bufs=1))
        wo_all = wo_pool.tile([128, 4, D], F32R)
        for jc in range(4):
            nc.sync.dma_start(wo_all[:, jc, :], woT_d[jc * 128:(jc + 1) * 128, :])
        y_pool = p3.enter_context(tc.tile_pool(name="ysb", bufs=4))
        ps3 = p3.enter_context(tc.tile_pool(name="ps3", bufs=4, space="PSUM"))
        for ttt in range(16):
            t0 = ttt * 128
            for ob in range(2):
                ps = ps3.tile([128, 512], F32, tag="y")
                for jc in range(4):
                    nc.tensor.matmul(ps, oT_all[:, jc, t0:t0 + 128],
                                     wo_all[:, jc, ob * 512:(ob + 1) * 512],
                                     start=(jc == 0), stop=(jc == 3))
                ysb = y_pool.tile([128, 512], F32, tag="ysb")
                nc.vector.tensor_copy(ysb, ps)
                nc.sync.dma_start(y_d[t0:t0 + 128, ob * 512:(ob + 1) * 512], ysb)


def build_module():
    nc = bacc.Bacc("TRN2", target_bir_lowering=False, debug=False,
                   num_devices=NCORES)
    xT_d = nc.dram_tensor("xT", [D, T], F32R, kind="ExternalInput").ap()
    wT_d = nc.dram_tensor("wT", [D, 3 * JL], F32R, kind="ExternalInput").ap()
    woT_d = nc.dram_tensor("woT", [JL, D], F32R, kind="ExternalInput").ap()
    pfx_d = nc.dram_tensor("pfx", [1, 1], F32, kind="ExternalInput").ap()
    y_d = nc.dram_tensor("y", [T, D], F32, kind="ExternalOutput").ap()
    with tile.TileContext(nc) as tc:
        with ExitStack() as ctx:
            _emit(ctx, tc, y_d, xT_d, wT_d, woT_d, pfx_d)
    nc.compile()
    return nc


_NC = None


def _get_nc():
    global _NC
    if _NC is None:
        _NC = build_module()
    return _NC


def shard_inputs(x, prefix_lengths, W_qkv, W_o):
    x = np.asarray(x, dtype=np.float32)
    W_qkv = np.asarray(W_qkv, dtype=np.float32)
    W_o = np.asarray(W_o, dtype=np.float32)
    pl = np.asarray(prefix_lengths)
    in_maps = []
    for c in range(NCORES):
        b, g = c // 2, c % 2
        W_loc = np.concatenate([
            W_qkv[JL * g:JL * (g + 1)],
            W_qkv[D + JL * g:D + JL * (g + 1)],
            W_qkv[2 * D + JL * g:2 * D + JL * (g + 1)],
        ], axis=0)
        p = float(min(max(int(pl[b]), 0), T))
        in_maps.append({
            "xT": np.ascontiguousarray(x[b].T),
            "wT": np.ascontiguousarray(W_loc.T),
            "woT": np.ascontiguousarray(W_o[:, JL * g:JL * (g + 1)].T),
            "pfx": np.array([[p]], dtype=np.float32),
        })
    return in_maps


def run(x, prefix_lengths, W_qkv, W_o, **kw):
    """Run the kernel; returns (y, BassKernelResults)."""
    nc = _get_nc()
    in_maps = shard_inputs(x, prefix_lengths, W_qkv, W_o)
    res = run_bass_kernel_spmd(nc, in_maps, core_ids=list(range(NCORES)), **kw)
    y = np.zeros((B, T, D), dtype=np.float32)
    for c in range(NCORES):
        y[c // 2] += res.results[c]["y"]
    return y, res


def kernel(x, prefix_lengths, W_qkv, W_o):
    y, _ = run(x, prefix_lengths, W_qkv, W_o)
    return y
